# revision 3
# baseline (speedup 1.0000x reference)
"""Cross-conditional GPT2 sparse attention block on 8 Trainium2 NeuronCores.

Sharding: core = (batch b in 0..3) x (head-group g in 0..1, 6 heads each).
Each core computes, for its (b, g):
  qT/kT = (Wq_g @ x_b^T + bq_g)  laid out [d_on_partitions, L]
  v     = x_b @ Wv_g^T + bv_g    natural layout [L, 6, 65] with a ones column
          per head so att@v also yields the softmax denominator for free.
  scores are computed *transposed* (sT[j, i]) so softmax needs no transpose:
  exp on ACT, multiplicative 0/1 mask (host-built, f16), then att@v with the
  exp-scores as the STATIONARY operand and v as moving, accumulating
  psY[i_tile][i, 6*65] — one PSUM bank per i-tile shared by all heads. The
  softmax denominator lands per-partition (col h*65+64), so normalization is
  a per-partition tensor_scalar multiply. y is then transposed via PE
  (identity matmul) into yT for the output projection with Wp[:, g]^T.
Host sums the two per-batch partials and adds bp.
"""

import sys

sys.path.insert(0, "/opt/trn_rl_repo")

from contextlib import ExitStack

import ml_dtypes
import numpy as np

import concourse.bacc as bacc
import concourse.bass as bass
import concourse.mybir as mybir
import concourse.tile as tile
from concourse.bass_utils import run_bass_kernel_spmd

# ---- problem constants (hardcoded per spec) ----
B = 4
T = 512
N = 8
C = 768
NHEAD = 12
L = 3 * T + 4 * N  # 1568
P = 128
G = C // 2  # 384 channels per head-group
NH = 6  # heads per core
D = 64  # head dim
ET = C // P  # 6 e-tiles (contraction of x @ W)
CT = G // P  # 3 c-tiles of the group's channels
NJT = (L + P - 1) // P  # 13 j tiles (12x128 + 32)
I_CHUNKS = [(0, 512), (512, 512), (1024, 512), (1536, 32)]
SCALE = 1.0 / 8.0  # 1/sqrt(64)

F32 = mybir.dt.float32
BF16 = mybir.dt.bfloat16
F16 = mybir.dt.float16

_NC = None  # cached compiled Bass program


def _jl(jt):
    return P if jt < NJT - 1 else L - (NJT - 1) * P  # 128 or 32


# (group) -> per-jt score interval (a, ln) over query index i.
# g0 = upper rows (i 0..512); g1 = lower rows (512..1024); g2 = torso+text.
def _grp_interval(g, jt):
    j0 = jt * P
    f0 = (jt % 4) * P if jt <= 11 else 0
    if g == 0:
        return (j0, 512 - j0) if jt <= 3 else None
    if g == 1:
        s = j0 if jt <= 3 else f0
        return (512 + s, 512 - s)
    s = j0 if jt <= 3 else f0
    return (1024 + s, 544 - s)


# mask kind per (group, jt): 'T1' | 'T2' | 'TXT' | None
def _grp_mask(g, jt):
    if jt == 12:
        return "TXT" if g in (1, 2) else None
    if g == 0:
        return "T1"
    if g == 1:
        return "T1" if jt <= 3 else "T2"
    return "T1" if jt <= 7 else "T2"


_GRP_ITS = {0: range(0, 4), 1: range(4, 8), 2: range(8, 13)}


def _build_program():
    nc = bacc.Bacc("TRN2", target_bir_lowering=False, debug=False)

    xT_d = nc.dram_tensor("xT", [C, L], F16, kind="ExternalInput")
    wq_d = nc.dram_tensor("wqT", [C, G], F16, kind="ExternalInput")
    wk_d = nc.dram_tensor("wkT", [C, G], F16, kind="ExternalInput")
    wv_d = nc.dram_tensor("wvT", [C, G], F16, kind="ExternalInput")
    wp_d = nc.dram_tensor("wpT", [G, C], F16, kind="ExternalInput")
    bq_d = nc.dram_tensor("bqP", [P, CT], F32, kind="ExternalInput")
    bk_d = nc.dram_tensor("bkP", [P, CT], F32, kind="ExternalInput")
    bv_d = nc.dram_tensor("bvB", [P, G], F32, kind="ExternalInput")
    maskd_d = nc.dram_tensor("maskD", [P, 2, P], F16, kind="ExternalInput")
    maskt_d = nc.dram_tensor("maskTxt", [32, 1024], F16, kind="ExternalInput")
    ident_d = nc.dram_tensor("ident", [P, P], F16, kind="ExternalInput")
    out_d = nc.dram_tensor("out_part", [L, C], F16, kind="ExternalOutput")

    with tile.TileContext(nc) as tc, ExitStack() as big:
        persist = big.enter_context(tc.tile_pool(name="persist", bufs=1))

        # persistent SBUF tensors
        qT = persist.tile([P, CT, L], F16, name="qT")
        kT = persist.tile([P, CT, L], F16, name="kT")
        v_ones = persist.tile([P, NJT, NH, D + 1], F16, name="v_ones")
        maskD = persist.tile([P, 2, P], F16, name="maskD_sb")
        maskTx = persist.tile([32, 1024], F16, name="maskTx_sb")
        yT = persist.tile([P, CT, L], F16, name="yT")
        wp_sb = persist.tile([P, CT, C], F16, name="wp_sb")
        ident = persist.tile([P, P], F16, name="ident_sb")
        bv_sb = persist.tile([P, G], F32, name="bv_sb")

        nc.sync.dma_start(maskD[:], maskd_d[:])
        nc.sync.dma_start(maskTx[:], maskt_d[:])
        nc.sync.dma_start(wp_sb[:], wp_d.rearrange("(ct p) n -> p ct n", p=P))
        nc.sync.dma_start(ident[:], ident_d[:])
        nc.sync.dma_start(bv_sb[:], bv_d[:])
        nc.gpsimd.memset(v_ones[:], 1.0)

        # ---------- Phase A: projections ----------
        with (
            tc.tile_pool(name="phA", bufs=1) as phA,
            tc.tile_pool(name="psA", bufs=2, space="PSUM") as psA,
        ):
            xT = phA.tile([P, ET, L], F16, name="xT_sb")
            wq_sb = phA.tile([P, ET, G], F16, name="wq_sb")
            wk_sb = phA.tile([P, ET, G], F16, name="wk_sb")
            wv_sb = phA.tile([P, ET, G], F16, name="wv_sb")
            bq_sb = phA.tile([P, CT], F32, name="bq_sb")
            bk_sb = phA.tile([P, CT], F32, name="bk_sb")

            nc.sync.dma_start(xT[:], xT_d.rearrange("(et p) i -> p et i", p=P))
            nc.sync.dma_start(wq_sb[:], wq_d.rearrange("(et p) m -> p et m", p=P))
            nc.sync.dma_start(wk_sb[:], wk_d.rearrange("(et p) m -> p et m", p=P))
            nc.sync.dma_start(wv_sb[:], wv_d.rearrange("(et p) m -> p et m", p=P))
            nc.sync.dma_start(bq_sb[:], bq_d[:])
            nc.sync.dma_start(bk_sb[:], bk_d[:])

            # qT / kT: out[c_tile, i] accumulated over e tiles
            for dst, w_sb, b_sb in ((qT, wq_sb, bq_sb), (kT, wk_sb, bk_sb)):
                for ct in range(CT):
                    for i0, ilen in I_CHUNKS:
                        ps = psA.tile([P, 512], F32, name="ps_qk", tag="ps_qk")
                        for et in range(ET):
                            nc.tensor.matmul(
                                ps[:, :ilen],
                                w_sb[:, et, ct * P : (ct + 1) * P],
                                xT[:, et, i0 : i0 + ilen],
                                start=(et == 0),
                                stop=(et == ET - 1),
                            )
                        nc.vector.tensor_scalar(
                            dst[:, ct, i0 : i0 + ilen],
                            ps[:, :ilen],
                            b_sb[:, ct : ct + 1],
                            None,
                            mybir.AluOpType.add,
                        )

            # v natural layout [i, 384] + bias, into the 65-strided f16 buffer
            for it in range(NJT):
                il = _jl(it)
                ps = psA.tile([P, G], F32, name="ps_v", tag="ps_v")
                for et in range(ET):
                    nc.tensor.matmul(
                        ps[:il, :],
                        xT[:, et, it * P : it * P + il],
                        wv_sb[:, et, :],
                        start=(et == 0),
                        stop=(et == ET - 1),
                    )
                nc.vector.tensor_tensor(
                    v_ones[:il, it, :, 0:D],
                    ps[:il, :].rearrange("p (h d) -> p h d", h=NH),
                    bv_sb[:il, :].rearrange("p (h d) -> p h d", h=NH),
                    mybir.AluOpType.add,
                )

        # ---------- Phase B+C: attention by row-group, proj interleaved ----------
        with (
            tc.tile_pool(name="phB", bufs=1) as phB,
            tc.tile_pool(name="phC", bufs=3) as phC,
            tc.tile_pool(name="psS", bufs=3, space="PSUM") as psS,
            tc.tile_pool(name="psY", bufs=5, space="PSUM") as psY_pool,
        ):
            for g in range(3):
                its = list(_GRP_ITS[g])
                jts = [jt for jt in range(NJT) if _grp_interval(g, jt) is not None]
                # one PSUM bank per i-tile: [i, 6 heads x (64 v-cols + den)]
                psY = {
                    it: psY_pool.tile([P, NH, D + 1], F32, name=f"ps_y{it}", tag="ps_y")
                    for it in its
                }
                started = set()
                for h in range(NH):
                    pof = D * (h % 2)
                    ct = h // 2
                    for jt in jts:
                        jl = _jl(jt)
                        a, ln = _grp_interval(g, jt)
                        chunks = [(a, min(ln, 512))]
                        if ln > 512:
                            chunks.append((a + 512, ln - 512))
                        for ca, cl in chunks:
                            ps_s = psS.tile([P, 512], F32, name="ps_s", tag="ps_s")
                            nc.tensor.matmul(
                                ps_s[:jl, :cl],
                                kT[pof : pof + D, ct, jt * P : jt * P + jl],
                                qT[pof : pof + D, ct, ca : ca + cl],
                                start=True,
                                stop=True,
                            )
                            pt = phB.tile([P, 512], F16, name="pT", tag="pT", bufs=16)
                            nc.scalar.activation(
                                pt[:jl, :cl],
                                ps_s[:jl, :cl],
                                mybir.ActivationFunctionType.Exp,
                                bias=0.0,
                                scale=SCALE,
                            )
                            mk = _grp_mask(g, jt)
                            if ca == a and mk in ("T1", "T2"):
                                nc.vector.tensor_tensor(
                                    pt[:jl, 0:P],
                                    pt[:jl, 0:P],
                                    maskD[:jl, 0 if mk == "T1" else 1, :],
                                    mybir.AluOpType.mult,
                                )
                            elif ca == a and mk == "TXT":
                                m0 = a - 512
                                nc.vector.tensor_tensor(
                                    pt[:jl, :cl],
                                    pt[:jl, :cl],
                                    maskTx[:jl, m0 : m0 + cl],
                                    mybir.AluOpType.mult,
                                )
                            # att@v: exp-scores stationary, v moving; one
                            # matmul per overlapped i-tile (128-aligned).
                            s0 = ca
                            while s0 < ca + cl:
                                it = s0 // P
                                e0 = min(ca + cl, (it + 1) * P)
                                sl = e0 - s0
                                last_jt = it if g == 0 else 12
                                nc.tensor.matmul(
                                    psY[it][0:sl, h, :],
                                    pt[:jl, s0 - ca : s0 - ca + sl],
                                    v_ones[:jl, jt, h, :],
                                    start=(it, h) not in started,
                                    stop=(jt == last_jt),
                                    skip_group_check=True,
                                )
                                started.add((it, h))
                                s0 = e0

                # finalize + output projection per i-tile
                for it in its:
                    il = _jl(it)
                    rcp = phB.tile([P, NH], F32, name="rcp", tag="rcp", bufs=4)
                    nc.vector.reciprocal_approx_fast(
                        out=rcp[:il, :], in_=psY[it][:il, :, D]
                    )
                    y_nrm = phB.tile([P, NH, D], F16, name="y_nrm", tag="y_nrm", bufs=4)
                    for h in range(NH):
                        nc.vector.tensor_scalar(
                            y_nrm[:il, h, :],
                            psY[it][:il, h, 0:D],
                            rcp[:il, h : h + 1],
                            None,
                            mybir.AluOpType.mult,
                        )
                    ps_t = psS.tile([P, CT, P], F16, name="ps_t", tag="ps_s")
                    for ct in range(CT):
                        nc.tensor.transpose(
                            ps_t[:, ct, :il],
                            y_nrm[:il, :, :].rearrange("p h d -> p (h d)")[
                                :, ct * P : (ct + 1) * P
                            ],
                            ident[:il, :il],
                        )
                    nc.scalar.activation(
                        yT[:, :, it * P : it * P + il],
                        ps_t[:, :, :il],
                        mybir.ActivationFunctionType.Copy,
                        bias=0.0,
                        scale=1.0,
                    )

                    o_sb = phC.tile([P, C], F16, name="o_sb", tag="o_sb")
                    for nch in range(2):
                        ps_o = psS.tile([P, 512], F32, name="ps_o", tag="ps_s")
                        for kt in range(CT):
                            nc.tensor.matmul(
                                ps_o[:il, :384],
                                yT[:, kt, it * P : it * P + il],
                                wp_sb[:, kt, nch * 384 : (nch + 1) * 384],
                                start=(kt == 0),
                                stop=(kt == CT - 1),
                                skip_group_check=True,
                            )
                        if nch == 0:
                            nc.vector.tensor_copy(
                                o_sb[:il, nch * 384 : (nch + 1) * 384], ps_o[:il, :384]
                            )
                        else:
                            nc.scalar.activation(
                                o_sb[:il, nch * 384 : (nch + 1) * 384],
                                ps_o[:il, :384],
                                mybir.ActivationFunctionType.Copy,
                                bias=0.0,
                                scale=1.0,
                            )
                    nc.sync.dma_start(out_d[it * P : it * P + il, :], o_sb[:il, :])

    nc.compile()
    return nc


def _build_mask_np(seg_starts, seg_ends):
    """True = masked. Mirrors reference._build_mask in numpy."""
    ML = 3 * T
    tril = np.tril(np.ones((T, T), dtype=bool))
    sl = np.tril(np.ones((T, T), dtype=bool), -1)
    m = np.zeros((L, L), dtype=bool)
    m[:ML, :ML] = True
    m[0:T, 0:T] = ~tril
    m[T : 2 * T, 0:T] = ~tril
    m[T : 2 * T, T : 2 * T] = ~sl
    m[T : 2 * T, 2 * T : 3 * T] = ~sl
    m[2 * T : 3 * T, 0:T] = ~tril
    m[2 * T : 3 * T, T : 2 * T] = ~tril
    m[2 * T : 3 * T, 2 * T : 3 * T] = ~sl
    m[:ML, ML:] = True
    frames = np.arange(T)[None, :, None]
    allowed = (frames >= seg_starts[:, None, :]) & (frames < seg_ends[:, None, :])
    mask = np.broadcast_to(m[None], (B, L, L)).copy()
    for row0, col_blocks in ((T, (0, 2, 3)), (2 * T, (1, 2, 3))):
        for j in col_blocks:
            c0 = ML + j * N
            mask[:, row0 : row0 + T, c0 : c0 + N] &= ~allowed
    return mask


def get_nc():
    global _NC
    if _NC is None:
        _NC = _build_program()
    return _NC


def make_in_maps(x, Wq, bq, Wk, bk, Wv, bv, Wp, bp, seg_starts, seg_ends):
    mask = _build_mask_np(np.asarray(seg_starts), np.asarray(seg_ends))
    r = np.arange(P)
    maskD = np.empty((P, 2, P), dtype=np.float16)
    maskD[:, 0, :] = (r[:, None] <= r[None, :]).astype(np.float16)  # tril.T
    maskD[:, 1, :] = (r[:, None] < r[None, :]).astype(np.float16)  # strict
    ident = np.eye(P, dtype=np.float16)
    in_maps = []
    for core in range(8):
        b, g = core // 2, core % 2
        gs = slice(g * G, (g + 1) * G)
        allowT = ~mask[b].T  # [j, i]
        maskTx = np.ascontiguousarray(
            allowT[1536:1568, 512:1536].astype(np.float16)
        )
        in_maps.append(
            {
                "xT": np.ascontiguousarray(x[b].T).astype(np.float16),
                "wqT": np.ascontiguousarray(Wq[gs, :].T).astype(np.float16),
                "wkT": np.ascontiguousarray(Wk[gs, :].T).astype(np.float16),
                "wvT": np.ascontiguousarray(Wv[gs, :].T).astype(np.float16),
                "wpT": np.ascontiguousarray(Wp[:, gs].T).astype(np.float16),
                "bqP": np.ascontiguousarray(bq[gs].reshape(CT, P).T),
                "bkP": np.ascontiguousarray(bk[gs].reshape(CT, P).T),
                "bvB": np.broadcast_to(bv[gs], (P, G)).copy(),
                "maskD": maskD,
                "maskTxt": maskTx,
                "ident": ident,
            }
        )
    return in_maps


def kernel(x, Wq, bq, Wk, bk, Wv, bv, Wp, bp, seg_starts, seg_ends, T_motion=None,
           N=None, _trace=False, **_unused):
    x = np.asarray(x, np.float32)
    args = [np.asarray(a, np.float32) for a in (Wq, bq, Wk, bk, Wv, bv, Wp, bp)]
    Wq, bq, Wk, bk, Wv, bv, Wp, bp = args
    nc = get_nc()
    in_maps = make_in_maps(x, Wq, bq, Wk, bk, Wv, bv, Wp, bp, seg_starts, seg_ends)
    res = run_bass_kernel_spmd(nc, in_maps, core_ids=list(range(8)), trace=_trace)
    parts = [r["out_part"] for r in res.results]
    y = np.empty((B, L, C), np.float32)
    for b in range(B):
        y[b] = parts[2 * b].astype(np.float32) + parts[2 * b + 1].astype(np.float32) + bp
    if _trace:
        kernel.last_results = res
    return y
